# revision 5
# baseline (speedup 1.0000x reference)
"""Trainium2 Bass kernel for nn_BigramLanguageModel_V2 (dense transformer fwd +
log-softmax CE loss), 8-core data-parallel SPMD.

Sharding: core c handles batch b=c//2, query-half h=c%2 (1024 of 2048 tokens).
Each core computes full-sequence K/V for its batch, its half's Q, causal
attention, the lm_head matmul over the full 32000 vocab for its 1024 tokens,
and fused log-softmax statistics (exp+row-sum on ScalarE). All matmuls bf16
with fp32 PSUM accumulation; lm_b applied exactly via a K=2 (hi+lo) ones-row
matmul. Causal masks are per-core *input data* so all 8 cores share one SPMD
program. Host does only sharding / casts / gathers and the final loss
reduction.
"""
import numpy as np
import ml_dtypes
from contextlib import ExitStack

import concourse.bass as bass
import concourse.tile as tile
from concourse import bacc, mybir
from concourse import library_config

BF16 = mybir.dt.bfloat16
F32 = mybir.dt.float32
I16 = mybir.dt.int16
AF = mybir.ActivationFunctionType

B, T, E, V = 4, 2048, 768, 32000
TQ = T // 2              # tokens per core
EC = E // 128            # 6 embedding chunks
VC = 500                 # vocab chunk width
NVC = V // VC            # 64 chunks
NQT = TQ // 128          # 8 query tiles per core
NST = T // 128           # 16 key tiles
SCALE = float(E) ** -0.5

_nbf = np.dtype(ml_dtypes.bfloat16)


def _build_program():
    nc = bacc.Bacc("TRN2", target_bir_lowering=False, debug=False)

    xe_d = nc.dram_tensor("xe", [128, EC, T], BF16, kind="ExternalInput")
    xqe_d = nc.dram_tensor("xqe", [128, EC, TQ], BF16, kind="ExternalInput")
    posf_d = nc.dram_tensor("posf", [128, EC, T], BF16, kind="ExternalInput")
    posq_d = nc.dram_tensor("posq", [128, EC, TQ], BF16, kind="ExternalInput")
    wq_d = nc.dram_tensor("wq", [128, EC, E], BF16, kind="ExternalInput")
    wk_d = nc.dram_tensor("wk", [128, EC, E], BF16, kind="ExternalInput")
    wv_d = nc.dram_tensor("wv", [128, EC, E], BF16, kind="ExternalInput")
    mask_d = nc.dram_tensor("mask", [NST, 128, TQ], BF16, kind="ExternalInput")
    b2_d = nc.dram_tensor("b2", [2, V], BF16, kind="ExternalInput")
    wtgt_d = nc.dram_tensor("wtgt", [128, EC, TQ], BF16, kind="ExternalInput")
    lmw_d = nc.dram_tensor("lmw", [128, EC, V], BF16, kind="ExternalInput")

    logits_d = nc.dram_tensor("logits", [TQ, V], F32, kind="ExternalOutput")
    stats_d = nc.dram_tensor("stats", [128, 16], F32, kind="ExternalOutput")

    with tile.TileContext(nc) as tc, ExitStack() as ctx:
        # ---- persistent pools ----
        pk = ctx.enter_context(tc.tile_pool(name="keep", bufs=1))
        kT = pk.tile([128, EC, T], BF16, tag="kT")
        qT = pk.tile([128, EC, TQ], BF16, tag="qT")
        v_sb = pk.tile([128, NST, E], BF16, tag="v")
        xaT = pk.tile([128, EC, TQ], BF16, tag="xaT")
        seacc = [pk.tile([128, NVC], F32, tag=f"seacc{qt}", name=f"seacc{qt}")
                 for qt in range(NQT)]
        ones2 = pk.tile([2, 128], BF16, tag="ones2")
        nc.vector.memset(ones2[:], 1.0)
        onesd = pk.tile([128, 1], BF16, tag="onesd")
        nc.vector.memset(onesd[:], 1.0)
        ones1f = pk.tile([1, 128], F32, tag="ones1f")
        nc.vector.memset(ones1f[:], 1.0)

        # ---- phase E+P: embed & QKV projections ----
        with tc.tile_pool(name="emb", bufs=1) as pe, \
             tc.tile_pool(name="psA", bufs=4, space="PSUM") as psA:
            xT = pe.tile([128, EC, T], BF16, tag="xT")
            nc.sync.dma_start(xT[:], xe_d.ap())
            xqT = pe.tile([128, EC, TQ], BF16, tag="xqT")
            nc.sync.dma_start(xqT[:], xqe_d.ap())
            posf = pe.tile([128, EC, T], BF16, tag="posf")
            nc.sync.dma_start(posf[:], posf_d.ap())
            posq = pe.tile([128, EC, TQ], BF16, tag="posq")
            nc.sync.dma_start(posq[:], posq_d.ap())
            nc.vector.tensor_add(xT[:], xT[:], posf[:])
            nc.vector.tensor_add(xqT[:], xqT[:], posq[:])

            wq = pe.tile([128, EC, E], BF16, tag="wq")
            nc.sync.dma_start(wq[:], wq_d.ap())
            wk = pe.tile([128, EC, E], BF16, tag="wk")
            nc.sync.dma_start(wk[:], wk_d.ap())
            wv = pe.tile([128, EC, E], BF16, tag="wv")
            nc.sync.dma_start(wv[:], wv_d.ap())

            # k^T [E, T] and q^T [E, TQ] in d-chunk layout
            for m in range(EC):
                for n in range(T // 512):
                    ps = psA.tile([128, 512], F32, tag="psA")
                    for c in range(EC):
                        nc.tensor.matmul(ps[:], lhsT=wk[:, c, m * 128:(m + 1) * 128],
                                         rhs=xT[:, c, n * 512:(n + 1) * 512],
                                         start=(c == 0), stop=(c == EC - 1))
                    nc.vector.tensor_copy(kT[:, m, n * 512:(n + 1) * 512], ps[:])
            for m in range(EC):
                for n in range(TQ // 512):
                    ps = psA.tile([128, 512], F32, tag="psA")
                    for c in range(EC):
                        nc.tensor.matmul(ps[:], lhsT=wq[:, c, m * 128:(m + 1) * 128],
                                         rhs=xqT[:, c, n * 512:(n + 1) * 512],
                                         start=(c == 0), stop=(c == EC - 1))
                    nc.vector.tensor_copy(qT[:, m, n * 512:(n + 1) * 512], ps[:])
            # v natural [T, E]
            for s in range(NST):
                for hf in range(2):
                    ps = psA.tile([128, 384], F32, tag="psAv")
                    for c in range(EC):
                        nc.tensor.matmul(ps[:], lhsT=xT[:, c, s * 128:(s + 1) * 128],
                                         rhs=wv[:, c, hf * 384:(hf + 1) * 384],
                                         start=(c == 0), stop=(c == EC - 1))
                    nc.vector.tensor_copy(v_sb[:, s, hf * 384:(hf + 1) * 384], ps[:])

        # ---- phase A: causal attention, [s, q] orientation ----
        with tc.tile_pool(name="att", bufs=3) as pa, \
             tc.tile_pool(name="attm", bufs=3) as pm, \
             tc.tile_pool(name="attr", bufs=2) as pr, \
             tc.tile_pool(name="psB", bufs=1, space="PSUM") as psB:
            for g in range(2):                       # q-chunk of 512
                q0 = g * 512
                # sigma needed iff 128*s <= max over cores of last query
                n_sig = 12 if g == 0 else 16
                att = [psB.tile([128, 512], F32, tag=f"att{i}", name=f"att{i}")
                       for i in range(EC)]
                den = psB.tile([1, 512], F32, tag="den")
                for s in range(n_sig):
                    st = psB.tile([128, 512], F32, tag="st")
                    for c in range(EC):
                        nc.tensor.matmul(st[:], lhsT=kT[:, c, s * 128:(s + 1) * 128],
                                         rhs=qT[:, c, q0:q0 + 512],
                                         start=(c == 0), stop=(c == EC - 1))
                    pT = pa.tile([128, 512], BF16, tag="pT")
                    nc.scalar.activation(pT[:], st[:], AF.Exp, scale=SCALE)
                    # mask unless fully-valid for every core (h=0 bound)
                    if not (128 * s + 127 <= q0):
                        mk = pm.tile([128, 512], BF16, tag="mk")
                        nc.sync.dma_start(mk[:], mask_d.ap()[s, :, q0:q0 + 512])
                        nc.vector.tensor_mul(pT[:], pT[:], mk[:])
                    first, last = (s == 0), (s == n_sig - 1)
                    for i in range(EC):
                        nc.tensor.matmul(att[i][:],
                                         lhsT=v_sb[:, s, i * 128:(i + 1) * 128],
                                         rhs=pT[:], start=first, stop=last)
                    nc.tensor.matmul(den[:], lhsT=onesd[:], rhs=pT[:],
                                     start=first, stop=last)
                # normalize: xaT[:, :, q-slice] = att / den (bcast over partitions)
                rec = pr.tile([1, 512], F32, tag="rec")
                nc.vector.reciprocal(rec[:], den[:])
                rb_ps = psB.tile([128, 512], F32, tag="st")
                nc.tensor.matmul(rb_ps[:], lhsT=ones1f[:], rhs=rec[:],
                                 start=True, stop=True)
                rb = pr.tile([128, 512], F32, tag="rb")
                nc.vector.tensor_copy(rb[:], rb_ps[:])
                for i in range(EC):
                    nc.vector.tensor_mul(xaT[:, i, q0:q0 + 512], att[i][:], rb[:])

        # ---- phase L: lm_head + fused CE stats ----
        with tc.tile_pool(name="lmw", bufs=2) as pw, \
             tc.tile_pool(name="lmo", bufs=4) as po, \
             tc.tile_pool(name="lms", bufs=2) as pescr, \
             tc.tile_pool(name="psC", bufs=8, space="PSUM") as psC:
            for vc in range(NVC):
                wt = pw.tile([128, EC, VC], BF16, tag="wt")
                nc.sync.dma_start(wt[:], lmw_d.ap()[:, :, vc * VC:(vc + 1) * VC])
                bt = pw.tile([2, VC], BF16, tag="bt")
                nc.sync.dma_start(bt[:], b2_d.ap()[:, vc * VC:(vc + 1) * VC])
                for qt in range(NQT):
                    ps = psC.tile([128, VC], F32, tag="lps")
                    nc.tensor.matmul(ps[:], lhsT=ones2[:], rhs=bt[:],
                                     start=True, stop=False)
                    for c in range(EC):
                        nc.tensor.matmul(ps[:],
                                         lhsT=xaT[:, c, qt * 128:(qt + 1) * 128],
                                         rhs=wt[:, c, :],
                                         start=False, stop=(c == EC - 1))
                    esc = pescr.tile([128, VC], BF16, tag="esc")
                    nc.scalar.activation(esc[:], ps[:], AF.Exp,
                                         accum_out=seacc[qt][:, vc:vc + 1])
                    lsb = po.tile([128, VC], F32, tag="lsb")
                    nc.vector.tensor_copy(lsb[:], ps[:])
                    nc.sync.dma_start(
                        logits_d.ap()[qt * 128:(qt + 1) * 128, vc * VC:(vc + 1) * VC],
                        lsb[:])

        # ---- phase S: stats assembly ----
        with tc.tile_pool(name="st", bufs=1) as pst:
            stats = pst.tile([128, 16], F32, tag="stats")
            nc.vector.memset(stats[:], 0.0)
            setot = pst.tile([128, NQT], F32, tag="setot")
            for qt in range(NQT):
                nc.vector.reduce_sum(setot[:, qt:qt + 1], seacc[qt][:],
                                     axis=mybir.AxisListType.X)
            nc.scalar.activation(stats[:, 0:NQT], setot[:], AF.Ln)
            wtg = pst.tile([128, EC, TQ], BF16, tag="wtg")
            nc.sync.dma_start(wtg[:], wtgt_d.ap())
            prod = pst.tile([128, TQ], F32, tag="prod")
            pacc = pst.tile([128, EC], F32, tag="pacc")
            for c in range(EC):
                nc.vector.tensor_mul(prod[:], xaT[:, c, :], wtg[:, c, :])
                nc.vector.reduce_sum(pacc[:, c:c + 1], prod[:],
                                     axis=mybir.AxisListType.X)
            nc.vector.reduce_sum(stats[:, 8:9], pacc[:], axis=mybir.AxisListType.X)
            nc.sync.dma_start(stats_d.ap(), stats[:])

    nc.compile()
    return nc


_NC_CACHE = None


def _get_nc():
    global _NC_CACHE
    if _NC_CACHE is None:
        _NC_CACHE = _build_program()
    return _NC_CACHE


def _wrap_idx(ix):
    """[n] int -> [128, n//16] int16 wrapped layout for dma_gather."""
    n = ix.shape[0]
    w = np.zeros((16, n // 16), dtype=np.int16)
    w[np.arange(n) % 16, np.arange(n) // 16] = ix.astype(np.int16)
    return np.tile(w, (8, 1))


def _chunked(w):
    """[E, N] -> [128, EC, N] with [p, c, n] = w[c*128+p, n]."""
    N = w.shape[1]
    return np.ascontiguousarray(w.reshape(EC, 128, N).transpose(1, 0, 2))


def make_in_maps(inputs):
    idx = np.asarray(inputs["idx"])
    target = np.asarray(inputs["target"])
    tok_emb = np.asarray(inputs["tok_emb"], dtype=np.float32)
    pos_emb = np.asarray(inputs["pos_emb"], dtype=np.float32)
    Wq = np.asarray(inputs["Wq"], dtype=np.float32)
    Wk = np.asarray(inputs["Wk"], dtype=np.float32)
    Wv = np.asarray(inputs["Wv"], dtype=np.float32)
    lm_W = np.asarray(inputs["lm_W"], dtype=np.float32)
    lm_b = np.asarray(inputs["lm_b"], dtype=np.float32)

    emb_bf = tok_emb.astype(_nbf)
    lmw_l = _chunked(lm_W.astype(_nbf))                      # [128, 6, V]
    wq_l = _chunked(Wq.astype(_nbf))
    wk_l = _chunked(Wk.astype(_nbf))
    wv_l = _chunked(Wv.astype(_nbf))
    b_hi = lm_b.astype(_nbf)
    b_lo = (lm_b - b_hi.astype(np.float32)).astype(_nbf)
    b2 = np.stack([b_hi, b_lo])                              # [2, V]
    posT = np.ascontiguousarray(
        pos_emb.astype(_nbf).reshape(T, EC, 128).transpose(2, 1, 0))  # [128,6,T]

    s_idx = 128 * np.arange(NST)[:, None, None] + np.arange(128)[None, :, None]

    in_maps, b_tgt_sums = [], []
    for c in range(8):
        b, h = c // 2, c % 2
        tgt = target[b, h * TQ:(h + 1) * TQ]
        mask = (s_idx <= (h * TQ + np.arange(TQ))[None, None, :]).astype(_nbf)
        xe = np.ascontiguousarray(
            emb_bf[idx[b]].reshape(T, EC, 128).transpose(2, 1, 0))
        in_maps.append({
            "xe": xe,
            "xqe": np.ascontiguousarray(xe[:, :, h * TQ:(h + 1) * TQ]),
            "posf": posT,
            "posq": np.ascontiguousarray(posT[:, :, h * TQ:(h + 1) * TQ]),
            "wq": wq_l, "wk": wk_l, "wv": wv_l,
            "mask": mask,
            "b2": b2,
            "wtgt": np.ascontiguousarray(lmw_l[:, :, tgt]),
            "lmw": lmw_l,
        })
        b_tgt_sums.append(float(b_hi[tgt].astype(np.float32).sum()
                                + b_lo[tgt].astype(np.float32).sum()))
    return in_maps, b_tgt_sums


def assemble(results, b_tgt_sums):
    logits = np.empty((B, T, V), dtype=np.float32)
    nll_sum = 0.0
    for c in range(8):
        b, h = c // 2, c % 2
        logits[b, h * TQ:(h + 1) * TQ] = results[c]["logits"]
        st = results[c]["stats"]
        nll_sum += float(st[:, 0:NQT].sum()) - float(st[:, 8].sum()) - b_tgt_sums[c]
    loss = np.float32(nll_sum / (B * T))
    return logits, loss


def kernel(**inputs):
    from concourse.bass_utils import run_bass_kernel_spmd
    nc = _get_nc()
    in_maps, b_tgt_sums = make_in_maps(inputs)
    res = run_bass_kernel_spmd(nc, in_maps, core_ids=list(range(8)))
    return assemble(res.results, b_tgt_sums)


# revision 6
# speedup vs baseline: 1.1020x; 1.1020x over previous
"""Trainium2 Bass kernel for nn_BigramLanguageModel_V2 (dense transformer fwd +
log-softmax CE loss), 8-core data-parallel SPMD.

Sharding: core c handles batch b=c//2, query-half h=c%2 (1024 of 2048 tokens).
Each core computes full-sequence K/V for its batch, its half's Q, causal
attention, the lm_head matmul over the full 32000 vocab for its 1024 tokens,
and fused log-softmax statistics (exp+row-sum on ScalarE). All matmuls bf16
with fp32 PSUM accumulation; lm_b applied exactly via a K=2 (hi+lo) ones-row
matmul. Causal masks are per-core *input data* so all 8 cores share one SPMD
program. Host does only sharding / casts / gathers and the final loss
reduction.
"""
import numpy as np
import ml_dtypes
from contextlib import ExitStack

import concourse.bass as bass
import concourse.tile as tile
from concourse import bacc, mybir
from concourse import library_config

BF16 = mybir.dt.bfloat16
F32 = mybir.dt.float32
I16 = mybir.dt.int16
AF = mybir.ActivationFunctionType

B, T, E, V = 4, 2048, 768, 32000
TQ = T // 2              # tokens per core
EC = E // 128            # 6 embedding chunks
VC = 500                 # vocab chunk width
NVC = V // VC            # 64 chunks
NQT = TQ // 128          # 8 query tiles per core
NST = T // 128           # 16 key tiles
SCALE = float(E) ** -0.5

_nbf = np.dtype(ml_dtypes.bfloat16)


def _build_program():
    nc = bacc.Bacc("TRN2", target_bir_lowering=False, debug=False)

    xe_d = nc.dram_tensor("xe", [128, EC, T], BF16, kind="ExternalInput")
    xqe_d = nc.dram_tensor("xqe", [128, EC, TQ], BF16, kind="ExternalInput")
    posf_d = nc.dram_tensor("posf", [128, EC, T], BF16, kind="ExternalInput")
    posq_d = nc.dram_tensor("posq", [128, EC, TQ], BF16, kind="ExternalInput")
    wq_d = nc.dram_tensor("wq", [128, EC, E], BF16, kind="ExternalInput")
    wk_d = nc.dram_tensor("wk", [128, EC, E], BF16, kind="ExternalInput")
    wv_d = nc.dram_tensor("wv", [128, EC, E], BF16, kind="ExternalInput")
    mask_d = nc.dram_tensor("mask", [NST, 128, TQ], BF16, kind="ExternalInput")
    bias_d = nc.dram_tensor("biasb", [128, V], F32, kind="ExternalInput")
    wtgt_d = nc.dram_tensor("wtgt", [128, EC, TQ], BF16, kind="ExternalInput")
    lmw_d = nc.dram_tensor("lmw", [128, EC, V], BF16, kind="ExternalInput")

    logits_d = nc.dram_tensor("logits", [TQ, V], F32, kind="ExternalOutput")
    stats_d = nc.dram_tensor("stats", [128, 16], F32, kind="ExternalOutput")

    with tile.TileContext(nc) as tc, ExitStack() as ctx:
        # ---- persistent pools ----
        pk = ctx.enter_context(tc.tile_pool(name="keep", bufs=1))
        kT = pk.tile([128, EC, T], BF16, tag="kT")
        qT = pk.tile([128, EC, TQ], BF16, tag="qT")
        v_sb = pk.tile([128, NST, E], BF16, tag="v")
        xaT = pk.tile([128, EC, TQ], BF16, tag="xaT")
        seacc = [pk.tile([128, NVC], F32, tag=f"seacc{qt}", name=f"seacc{qt}")
                 for qt in range(NQT)]
        onesd = pk.tile([128, 1], BF16, tag="onesd")
        nc.vector.memset(onesd[:], 1.0)
        ones1f = pk.tile([1, 128], F32, tag="ones1f")
        nc.vector.memset(ones1f[:], 1.0)

        # ---- phase E+P: embed & QKV projections ----
        with tc.tile_pool(name="emb", bufs=1) as pe, \
             tc.tile_pool(name="psA", bufs=4, space="PSUM") as psA:
            xT = pe.tile([128, EC, T], BF16, tag="xT")
            nc.sync.dma_start(xT[:], xe_d.ap())
            xqT = pe.tile([128, EC, TQ], BF16, tag="xqT")
            nc.sync.dma_start(xqT[:], xqe_d.ap())
            posf = pe.tile([128, EC, T], BF16, tag="posf")
            nc.sync.dma_start(posf[:], posf_d.ap())
            posq = pe.tile([128, EC, TQ], BF16, tag="posq")
            nc.sync.dma_start(posq[:], posq_d.ap())
            nc.vector.tensor_add(xT[:], xT[:], posf[:])
            nc.vector.tensor_add(xqT[:], xqT[:], posq[:])

            wq = pe.tile([128, EC, E], BF16, tag="wq")
            nc.sync.dma_start(wq[:], wq_d.ap())
            wk = pe.tile([128, EC, E], BF16, tag="wk")
            nc.sync.dma_start(wk[:], wk_d.ap())
            wv = pe.tile([128, EC, E], BF16, tag="wv")
            nc.sync.dma_start(wv[:], wv_d.ap())

            # k^T [E, T] and q^T [E, TQ] in d-chunk layout
            for m in range(EC):
                for n in range(T // 512):
                    ps = psA.tile([128, 512], F32, tag="psA")
                    for c in range(EC):
                        nc.tensor.matmul(ps[:], lhsT=wk[:, c, m * 128:(m + 1) * 128],
                                         rhs=xT[:, c, n * 512:(n + 1) * 512],
                                         start=(c == 0), stop=(c == EC - 1))
                    nc.vector.tensor_copy(kT[:, m, n * 512:(n + 1) * 512], ps[:])
            for m in range(EC):
                for n in range(TQ // 512):
                    ps = psA.tile([128, 512], F32, tag="psA")
                    for c in range(EC):
                        nc.tensor.matmul(ps[:], lhsT=wq[:, c, m * 128:(m + 1) * 128],
                                         rhs=xqT[:, c, n * 512:(n + 1) * 512],
                                         start=(c == 0), stop=(c == EC - 1))
                    nc.vector.tensor_copy(qT[:, m, n * 512:(n + 1) * 512], ps[:])
            # v natural [T, E]
            for s in range(NST):
                for hf in range(2):
                    ps = psA.tile([128, 384], F32, tag="psAv")
                    for c in range(EC):
                        nc.tensor.matmul(ps[:], lhsT=xT[:, c, s * 128:(s + 1) * 128],
                                         rhs=wv[:, c, hf * 384:(hf + 1) * 384],
                                         start=(c == 0), stop=(c == EC - 1))
                    nc.vector.tensor_copy(v_sb[:, s, hf * 384:(hf + 1) * 384], ps[:])

        # ---- phase A: causal attention, [s, q] orientation ----
        with tc.tile_pool(name="att", bufs=3) as pa, \
             tc.tile_pool(name="attm", bufs=3) as pm, \
             tc.tile_pool(name="attr", bufs=2) as pr, \
             tc.tile_pool(name="psB", bufs=1, space="PSUM") as psB:
            for g in range(2):                       # q-chunk of 512
                q0 = g * 512
                # sigma needed iff 128*s <= max over cores of last query
                n_sig = 12 if g == 0 else 16
                att = [psB.tile([128, 512], F32, tag=f"att{i}", name=f"att{i}")
                       for i in range(EC)]
                den = psB.tile([1, 512], F32, tag="den")
                for s in range(n_sig):
                    st = psB.tile([128, 512], F32, tag="st")
                    for c in range(EC):
                        nc.tensor.matmul(st[:], lhsT=kT[:, c, s * 128:(s + 1) * 128],
                                         rhs=qT[:, c, q0:q0 + 512],
                                         start=(c == 0), stop=(c == EC - 1))
                    pT = pa.tile([128, 512], BF16, tag="pT")
                    nc.scalar.activation(pT[:], st[:], AF.Exp, scale=SCALE)
                    # mask unless fully-valid for every core (h=0 bound)
                    if not (128 * s + 127 <= q0):
                        mk = pm.tile([128, 512], BF16, tag="mk")
                        nc.sync.dma_start(mk[:], mask_d.ap()[s, :, q0:q0 + 512])
                        nc.vector.tensor_mul(pT[:], pT[:], mk[:])
                    first, last = (s == 0), (s == n_sig - 1)
                    for i in range(EC):
                        nc.tensor.matmul(att[i][:],
                                         lhsT=v_sb[:, s, i * 128:(i + 1) * 128],
                                         rhs=pT[:], start=first, stop=last)
                    nc.tensor.matmul(den[:], lhsT=onesd[:], rhs=pT[:],
                                     start=first, stop=last)
                # normalize: xaT[:, :, q-slice] = att / den (bcast over partitions)
                rec = pr.tile([1, 512], F32, tag="rec")
                nc.vector.reciprocal(rec[:], den[:])
                rb_ps = psB.tile([128, 512], F32, tag="st")
                nc.tensor.matmul(rb_ps[:], lhsT=ones1f[:], rhs=rec[:],
                                 start=True, stop=True)
                rb = pr.tile([128, 512], F32, tag="rb")
                nc.vector.tensor_copy(rb[:], rb_ps[:])
                for i in range(EC):
                    nc.vector.tensor_mul(xaT[:, i, q0:q0 + 512], att[i][:], rb[:])

        # ---- phase L: lm_head + fused CE stats ----
        with tc.tile_pool(name="lmw", bufs=2) as pw, \
             tc.tile_pool(name="lmo", bufs=4) as po, \
             tc.tile_pool(name="lms", bufs=2) as pescr, \
             tc.tile_pool(name="psC", bufs=8, space="PSUM") as psC:
            for vc in range(NVC):
                wt = pw.tile([128, EC, VC], BF16, tag="wt")
                nc.sync.dma_start(wt[:], lmw_d.ap()[:, :, vc * VC:(vc + 1) * VC])
                bt = pw.tile([128, VC], F32, tag="bt")
                nc.sync.dma_start(bt[:], bias_d.ap()[:, vc * VC:(vc + 1) * VC])
                for qt in range(NQT):
                    ps = psC.tile([128, VC], F32, tag="lps")
                    for c in range(EC):
                        nc.tensor.matmul(ps[:],
                                         lhsT=xaT[:, c, qt * 128:(qt + 1) * 128],
                                         rhs=wt[:, c, :],
                                         start=(c == 0), stop=(c == EC - 1))
                    lsb = po.tile([128, VC], F32, tag="lsb")
                    nc.vector.tensor_add(lsb[:], ps[:], bt[:])
                    esc = pescr.tile([128, VC], BF16, tag="esc")
                    nc.scalar.activation(esc[:], lsb[:], AF.Exp,
                                         accum_out=seacc[qt][:, vc:vc + 1])
                    nc.sync.dma_start(
                        logits_d.ap()[qt * 128:(qt + 1) * 128, vc * VC:(vc + 1) * VC],
                        lsb[:])

        # ---- phase S: stats assembly ----
        with tc.tile_pool(name="st", bufs=1) as pst:
            stats = pst.tile([128, 16], F32, tag="stats")
            nc.vector.memset(stats[:], 0.0)
            setot = pst.tile([128, NQT], F32, tag="setot")
            for qt in range(NQT):
                nc.vector.reduce_sum(setot[:, qt:qt + 1], seacc[qt][:],
                                     axis=mybir.AxisListType.X)
            nc.scalar.activation(stats[:, 0:NQT], setot[:], AF.Ln)
            wtg = pst.tile([128, EC, TQ], BF16, tag="wtg")
            nc.sync.dma_start(wtg[:], wtgt_d.ap())
            prod = pst.tile([128, TQ], F32, tag="prod")
            pacc = pst.tile([128, EC], F32, tag="pacc")
            for c in range(EC):
                nc.vector.tensor_mul(prod[:], xaT[:, c, :], wtg[:, c, :])
                nc.vector.reduce_sum(pacc[:, c:c + 1], prod[:],
                                     axis=mybir.AxisListType.X)
            nc.vector.reduce_sum(stats[:, 8:9], pacc[:], axis=mybir.AxisListType.X)
            nc.sync.dma_start(stats_d.ap(), stats[:])

    nc.compile()
    return nc


_NC_CACHE = None


def _get_nc():
    global _NC_CACHE
    if _NC_CACHE is None:
        _NC_CACHE = _build_program()
    return _NC_CACHE


def _wrap_idx(ix):
    """[n] int -> [128, n//16] int16 wrapped layout for dma_gather."""
    n = ix.shape[0]
    w = np.zeros((16, n // 16), dtype=np.int16)
    w[np.arange(n) % 16, np.arange(n) // 16] = ix.astype(np.int16)
    return np.tile(w, (8, 1))


def _chunked(w):
    """[E, N] -> [128, EC, N] with [p, c, n] = w[c*128+p, n]."""
    N = w.shape[1]
    return np.ascontiguousarray(w.reshape(EC, 128, N).transpose(1, 0, 2))


def make_in_maps(inputs):
    idx = np.asarray(inputs["idx"])
    target = np.asarray(inputs["target"])
    tok_emb = np.asarray(inputs["tok_emb"], dtype=np.float32)
    pos_emb = np.asarray(inputs["pos_emb"], dtype=np.float32)
    Wq = np.asarray(inputs["Wq"], dtype=np.float32)
    Wk = np.asarray(inputs["Wk"], dtype=np.float32)
    Wv = np.asarray(inputs["Wv"], dtype=np.float32)
    lm_W = np.asarray(inputs["lm_W"], dtype=np.float32)
    lm_b = np.asarray(inputs["lm_b"], dtype=np.float32)

    emb_bf = tok_emb.astype(_nbf)
    lmw_l = _chunked(lm_W.astype(_nbf))                      # [128, 6, V]
    wq_l = _chunked(Wq.astype(_nbf))
    wk_l = _chunked(Wk.astype(_nbf))
    wv_l = _chunked(Wv.astype(_nbf))
    bias_bc = np.ascontiguousarray(
        np.broadcast_to(lm_b.astype(np.float32), (128, V)))
    posT = np.ascontiguousarray(
        pos_emb.astype(_nbf).reshape(T, EC, 128).transpose(2, 1, 0))  # [128,6,T]

    s_idx = 128 * np.arange(NST)[:, None, None] + np.arange(128)[None, :, None]

    in_maps, b_tgt_sums = [], []
    for c in range(8):
        b, h = c // 2, c % 2
        tgt = target[b, h * TQ:(h + 1) * TQ]
        mask = (s_idx <= (h * TQ + np.arange(TQ))[None, None, :]).astype(_nbf)
        xe = np.ascontiguousarray(
            emb_bf[idx[b]].reshape(T, EC, 128).transpose(2, 1, 0))
        in_maps.append({
            "xe": xe,
            "xqe": np.ascontiguousarray(xe[:, :, h * TQ:(h + 1) * TQ]),
            "posf": posT,
            "posq": np.ascontiguousarray(posT[:, :, h * TQ:(h + 1) * TQ]),
            "wq": wq_l, "wk": wk_l, "wv": wv_l,
            "mask": mask,
            "biasb": bias_bc,
            "wtgt": np.ascontiguousarray(lmw_l[:, :, tgt]),
            "lmw": lmw_l,
        })
        b_tgt_sums.append(float(lm_b[tgt].astype(np.float32).sum()))
    return in_maps, b_tgt_sums


def assemble(results, b_tgt_sums):
    logits = np.empty((B, T, V), dtype=np.float32)
    nll_sum = 0.0
    for c in range(8):
        b, h = c // 2, c % 2
        logits[b, h * TQ:(h + 1) * TQ] = results[c]["logits"]
        st = results[c]["stats"]
        nll_sum += float(st[:, 0:NQT].sum()) - float(st[:, 8].sum()) - b_tgt_sums[c]
    loss = np.float32(nll_sum / (B * T))
    return logits, loss


def kernel(**inputs):
    from concourse.bass_utils import run_bass_kernel_spmd
    nc = _get_nc()
    in_maps, b_tgt_sums = make_in_maps(inputs)
    res = run_bass_kernel_spmd(nc, in_maps, core_ids=list(range(8)))
    return assemble(res.results, b_tgt_sums)


# revision 7
# speedup vs baseline: 1.1805x; 1.0713x over previous
"""Trainium2 Bass kernel for nn_BigramLanguageModel_V2 (dense transformer fwd +
log-softmax CE loss), 8-core data-parallel SPMD.

Sharding: core c handles batch b=c//2, query-half h=c%2 (1024 of 2048 tokens).
Each core computes full-sequence K/V for its batch, its half's Q, causal
attention, the lm_head matmul over the full 32000 vocab for its 1024 tokens,
and fused log-softmax statistics (exp+row-sum on ScalarE). All matmuls bf16
with fp32 PSUM accumulation; lm_b applied exactly via a K=2 (hi+lo) ones-row
matmul. Causal masks are per-core *input data* so all 8 cores share one SPMD
program. Host does only sharding / casts / gathers and the final loss
reduction.
"""
import numpy as np
import ml_dtypes
from contextlib import ExitStack

import concourse.bass as bass
import concourse.tile as tile
from concourse import bacc, mybir
from concourse import library_config

BF16 = mybir.dt.bfloat16
F32 = mybir.dt.float32
I16 = mybir.dt.int16
AF = mybir.ActivationFunctionType

B, T, E, V = 4, 2048, 768, 32000
TQ = T // 2              # tokens per core
EC = E // 128            # 6 embedding chunks
VC = 500                 # vocab chunk width
NVC = V // VC            # 64 chunks
NQT = TQ // 128          # 8 query tiles per core
NST = T // 128           # 16 key tiles
SCALE = float(E) ** -0.5

_nbf = np.dtype(ml_dtypes.bfloat16)


def _build_program():
    nc = bacc.Bacc("TRN2", target_bir_lowering=False, debug=False)

    xe_d = nc.dram_tensor("xe", [128, EC, T], BF16, kind="ExternalInput")
    xqe_d = nc.dram_tensor("xqe", [128, EC, TQ], BF16, kind="ExternalInput")
    posf_d = nc.dram_tensor("posf", [128, EC, T], BF16, kind="ExternalInput")
    posq_d = nc.dram_tensor("posq", [128, EC, TQ], BF16, kind="ExternalInput")
    wq_d = nc.dram_tensor("wq", [128, EC, E], BF16, kind="ExternalInput")
    wk_d = nc.dram_tensor("wk", [128, EC, E], BF16, kind="ExternalInput")
    wv_d = nc.dram_tensor("wv", [128, EC, E], BF16, kind="ExternalInput")
    mask_d = nc.dram_tensor("mask", [NST, 128, TQ], BF16, kind="ExternalInput")
    bias_d = nc.dram_tensor("biasb", [128, V], F32, kind="ExternalInput")
    wtgt_d = nc.dram_tensor("wtgt", [128, EC, TQ], BF16, kind="ExternalInput")
    lmw_d = nc.dram_tensor("lmw", [128, EC, V], BF16, kind="ExternalInput")

    logits_d = nc.dram_tensor("logits", [TQ, V], F32, kind="ExternalOutput")
    stats_d = nc.dram_tensor("stats", [128, 16], F32, kind="ExternalOutput")

    with tile.TileContext(nc) as tc, ExitStack() as ctx:
        # ---- persistent pools ----
        pk = ctx.enter_context(tc.tile_pool(name="keep", bufs=1))
        kT = pk.tile([128, EC, T], BF16, tag="kT")
        qT = pk.tile([128, EC, TQ], BF16, tag="qT")
        v_sb = pk.tile([128, NST, E], BF16, tag="v")
        xaT = pk.tile([128, EC, TQ], BF16, tag="xaT")
        seacc = [pk.tile([128, NVC], F32, tag=f"seacc{qt}", name=f"seacc{qt}")
                 for qt in range(NQT)]
        onesd = pk.tile([128, 1], BF16, tag="onesd")
        nc.vector.memset(onesd[:], 1.0)
        ones1f = pk.tile([1, 128], F32, tag="ones1f")
        nc.vector.memset(ones1f[:], 1.0)

        # ---- phase E+P: embed & QKV projections ----
        with tc.tile_pool(name="emb", bufs=1) as pe, \
             tc.tile_pool(name="psA", bufs=4, space="PSUM") as psA:
            xT = pe.tile([128, EC, T], BF16, tag="xT")
            posf = pe.tile([128, EC, T], BF16, tag="posf")
            xqT = pe.tile([128, EC, TQ], BF16, tag="xqT")
            posq = pe.tile([128, EC, TQ], BF16, tag="posq")
            for n in range(T // 512):
                sl = slice(n * 512, (n + 1) * 512)
                nc.sync.dma_start(xT[:, :, sl], xe_d.ap()[:, :, sl])
                nc.sync.dma_start(posf[:, :, sl], posf_d.ap()[:, :, sl])
                nc.vector.tensor_add(xT[:, :, sl], xT[:, :, sl], posf[:, :, sl])
            for n in range(TQ // 512):
                sl = slice(n * 512, (n + 1) * 512)
                nc.sync.dma_start(xqT[:, :, sl], xqe_d.ap()[:, :, sl])
                nc.sync.dma_start(posq[:, :, sl], posq_d.ap()[:, :, sl])
                nc.vector.tensor_add(xqT[:, :, sl], xqT[:, :, sl], posq[:, :, sl])

            wq = pe.tile([128, EC, E], BF16, tag="wq")
            nc.sync.dma_start(wq[:], wq_d.ap())
            wk = pe.tile([128, EC, E], BF16, tag="wk")
            nc.sync.dma_start(wk[:], wk_d.ap())
            wv = pe.tile([128, EC, E], BF16, tag="wv")
            nc.sync.dma_start(wv[:], wv_d.ap())

            # k^T [E, T] and q^T [E, TQ] in d-chunk layout; v natural [T, E]
            for n in range(T // 512):
                for m in range(EC):
                    ps = psA.tile([128, 512], F32, tag="psA")
                    for c in range(EC):
                        nc.tensor.matmul(ps[:], lhsT=wk[:, c, m * 128:(m + 1) * 128],
                                         rhs=xT[:, c, n * 512:(n + 1) * 512],
                                         start=(c == 0), stop=(c == EC - 1))
                    nc.vector.tensor_copy(kT[:, m, n * 512:(n + 1) * 512], ps[:])
                for s in range(4 * n, 4 * n + 4):
                    for hf in range(2):
                        ps = psA.tile([128, 384], F32, tag="psAv")
                        for c in range(EC):
                            nc.tensor.matmul(ps[:],
                                             lhsT=xT[:, c, s * 128:(s + 1) * 128],
                                             rhs=wv[:, c, hf * 384:(hf + 1) * 384],
                                             start=(c == 0), stop=(c == EC - 1))
                        nc.vector.tensor_copy(v_sb[:, s, hf * 384:(hf + 1) * 384],
                                              ps[:])
            for n in range(TQ // 512):
                for m in range(EC):
                    ps = psA.tile([128, 512], F32, tag="psA")
                    for c in range(EC):
                        nc.tensor.matmul(ps[:], lhsT=wq[:, c, m * 128:(m + 1) * 128],
                                         rhs=xqT[:, c, n * 512:(n + 1) * 512],
                                         start=(c == 0), stop=(c == EC - 1))
                    nc.vector.tensor_copy(qT[:, m, n * 512:(n + 1) * 512], ps[:])

        # ---- phase A: causal attention, [s, q] orientation ----
        with tc.tile_pool(name="att", bufs=3) as pa, \
             tc.tile_pool(name="attm", bufs=3) as pm, \
             tc.tile_pool(name="attr", bufs=2) as pr, \
             tc.tile_pool(name="psB", bufs=1, space="PSUM") as psB:
            for g in range(2):                       # q-chunk of 512
                q0 = g * 512
                # sigma needed iff 128*s <= max over cores of last query
                n_sig = 12 if g == 0 else 16
                att = [psB.tile([128, 512], F32, tag=f"att{i}", name=f"att{i}")
                       for i in range(EC)]
                den = psB.tile([1, 512], F32, tag="den")
                for s in range(n_sig):
                    st = psB.tile([128, 512], F32, tag="st")
                    for c in range(EC):
                        nc.tensor.matmul(st[:], lhsT=kT[:, c, s * 128:(s + 1) * 128],
                                         rhs=qT[:, c, q0:q0 + 512],
                                         start=(c == 0), stop=(c == EC - 1))
                    pT = pa.tile([128, 512], BF16, tag="pT")
                    nc.scalar.activation(pT[:], st[:], AF.Exp, scale=SCALE)
                    # mask unless fully-valid for every core (h=0 bound)
                    if not (128 * s + 127 <= q0):
                        mk = pm.tile([128, 512], BF16, tag="mk")
                        nc.sync.dma_start(mk[:], mask_d.ap()[s, :, q0:q0 + 512])
                        nc.vector.tensor_mul(pT[:], pT[:], mk[:])
                    first, last = (s == 0), (s == n_sig - 1)
                    for i in range(EC):
                        nc.tensor.matmul(att[i][:],
                                         lhsT=v_sb[:, s, i * 128:(i + 1) * 128],
                                         rhs=pT[:], start=first, stop=last)
                    nc.tensor.matmul(den[:], lhsT=onesd[:], rhs=pT[:],
                                     start=first, stop=last)
                # normalize: xaT[:, :, q-slice] = att / den (bcast over partitions)
                rec = pr.tile([1, 512], F32, tag="rec")
                nc.vector.reciprocal(rec[:], den[:])
                rb_ps = psB.tile([128, 512], F32, tag="st")
                nc.tensor.matmul(rb_ps[:], lhsT=ones1f[:], rhs=rec[:],
                                 start=True, stop=True)
                rb = pr.tile([128, 512], F32, tag="rb")
                nc.vector.tensor_copy(rb[:], rb_ps[:])
                for i in range(EC):
                    nc.vector.tensor_mul(xaT[:, i, q0:q0 + 512], att[i][:], rb[:])

        # ---- phase L: lm_head + fused CE stats ----
        with tc.tile_pool(name="lmw", bufs=3) as pw, \
             tc.tile_pool(name="lmo", bufs=6) as po, \
             tc.tile_pool(name="lms", bufs=3) as pescr, \
             tc.tile_pool(name="psC", bufs=8, space="PSUM") as psC, \
             tc.tile_pool(name="st", bufs=1) as pst:
            stats = pst.tile([128, 16], F32, tag="stats")
            nc.vector.memset(stats[:], 0.0)
            wtg = pst.tile([128, EC, TQ], BF16, tag="wtg")
            nc.sync.dma_start(wtg[:], wtgt_d.ap())
            prod = pst.tile([128, TQ], F32, tag="prod")
            pacc = pst.tile([128, EC], F32, tag="pacc")
            for c in range(EC):
                nc.vector.tensor_mul(prod[:], xaT[:, c, :], wtg[:, c, :])
                nc.vector.reduce_sum(pacc[:, c:c + 1], prod[:],
                                     axis=mybir.AxisListType.X)
            nc.vector.reduce_sum(stats[:, 8:9], pacc[:], axis=mybir.AxisListType.X)
            for vc in range(NVC):
                wt = pw.tile([128, EC, VC], BF16, tag="wt")
                nc.sync.dma_start(wt[:], lmw_d.ap()[:, :, vc * VC:(vc + 1) * VC])
                bt = pw.tile([128, VC], F32, tag="bt")
                nc.sync.dma_start(bt[:], bias_d.ap()[:, vc * VC:(vc + 1) * VC])
                for qt in range(NQT):
                    ps = psC.tile([128, VC], F32, tag="lps")
                    for c in range(EC):
                        nc.tensor.matmul(ps[:],
                                         lhsT=xaT[:, c, qt * 128:(qt + 1) * 128],
                                         rhs=wt[:, c, :],
                                         start=(c == 0), stop=(c == EC - 1))
                    lsb = po.tile([128, VC], F32, tag="lsb")
                    nc.vector.tensor_add(lsb[:], ps[:], bt[:])
                    esc = pescr.tile([128, VC], BF16, tag="esc")
                    nc.scalar.activation(esc[:], lsb[:], AF.Exp,
                                         accum_out=seacc[qt][:, vc:vc + 1])
                    nc.sync.dma_start(
                        logits_d.ap()[qt * 128:(qt + 1) * 128, vc * VC:(vc + 1) * VC],
                        lsb[:])

            # stats tail: logsumexp per token + DMA out
            setot = pst.tile([128, NQT], F32, tag="setot")
            for qt in range(NQT):
                nc.vector.reduce_sum(setot[:, qt:qt + 1], seacc[qt][:],
                                     axis=mybir.AxisListType.X)
            nc.scalar.activation(stats[:, 0:NQT], setot[:], AF.Ln)
            nc.sync.dma_start(stats_d.ap(), stats[:])

    nc.compile()
    return nc


_NC_CACHE = None


def _get_nc():
    global _NC_CACHE
    if _NC_CACHE is None:
        _NC_CACHE = _build_program()
    return _NC_CACHE


def _wrap_idx(ix):
    """[n] int -> [128, n//16] int16 wrapped layout for dma_gather."""
    n = ix.shape[0]
    w = np.zeros((16, n // 16), dtype=np.int16)
    w[np.arange(n) % 16, np.arange(n) // 16] = ix.astype(np.int16)
    return np.tile(w, (8, 1))


def _chunked(w):
    """[E, N] -> [128, EC, N] with [p, c, n] = w[c*128+p, n]."""
    N = w.shape[1]
    return np.ascontiguousarray(w.reshape(EC, 128, N).transpose(1, 0, 2))


def make_in_maps(inputs):
    idx = np.asarray(inputs["idx"])
    target = np.asarray(inputs["target"])
    tok_emb = np.asarray(inputs["tok_emb"], dtype=np.float32)
    pos_emb = np.asarray(inputs["pos_emb"], dtype=np.float32)
    Wq = np.asarray(inputs["Wq"], dtype=np.float32)
    Wk = np.asarray(inputs["Wk"], dtype=np.float32)
    Wv = np.asarray(inputs["Wv"], dtype=np.float32)
    lm_W = np.asarray(inputs["lm_W"], dtype=np.float32)
    lm_b = np.asarray(inputs["lm_b"], dtype=np.float32)

    emb_bf = tok_emb.astype(_nbf)
    lmw_l = _chunked(lm_W.astype(_nbf))                      # [128, 6, V]
    wq_l = _chunked(Wq.astype(_nbf))
    wk_l = _chunked(Wk.astype(_nbf))
    wv_l = _chunked(Wv.astype(_nbf))
    bias_bc = np.ascontiguousarray(
        np.broadcast_to(lm_b.astype(np.float32), (128, V)))
    posT = np.ascontiguousarray(
        pos_emb.astype(_nbf).reshape(T, EC, 128).transpose(2, 1, 0))  # [128,6,T]

    s_idx = 128 * np.arange(NST)[:, None, None] + np.arange(128)[None, :, None]

    in_maps, b_tgt_sums = [], []
    for c in range(8):
        b, h = c // 2, c % 2
        tgt = target[b, h * TQ:(h + 1) * TQ]
        mask = (s_idx <= (h * TQ + np.arange(TQ))[None, None, :]).astype(_nbf)
        xe = np.ascontiguousarray(
            emb_bf[idx[b]].reshape(T, EC, 128).transpose(2, 1, 0))
        in_maps.append({
            "xe": xe,
            "xqe": np.ascontiguousarray(xe[:, :, h * TQ:(h + 1) * TQ]),
            "posf": posT,
            "posq": np.ascontiguousarray(posT[:, :, h * TQ:(h + 1) * TQ]),
            "wq": wq_l, "wk": wk_l, "wv": wv_l,
            "mask": mask,
            "biasb": bias_bc,
            "wtgt": np.ascontiguousarray(lmw_l[:, :, tgt]),
            "lmw": lmw_l,
        })
        b_tgt_sums.append(float(lm_b[tgt].astype(np.float32).sum()))
    return in_maps, b_tgt_sums


def assemble(results, b_tgt_sums):
    logits = np.empty((B, T, V), dtype=np.float32)
    nll_sum = 0.0
    for c in range(8):
        b, h = c // 2, c % 2
        logits[b, h * TQ:(h + 1) * TQ] = results[c]["logits"]
        st = results[c]["stats"]
        nll_sum += float(st[:, 0:NQT].sum()) - float(st[:, 8].sum()) - b_tgt_sums[c]
    loss = np.float32(nll_sum / (B * T))
    return logits, loss


def kernel(**inputs):
    from concourse.bass_utils import run_bass_kernel_spmd
    nc = _get_nc()
    in_maps, b_tgt_sums = make_in_maps(inputs)
    res = run_bass_kernel_spmd(nc, in_maps, core_ids=list(range(8)))
    return assemble(res.results, b_tgt_sums)


# revision 8
# speedup vs baseline: 1.2553x; 1.0633x over previous
"""Trainium2 Bass kernel for nn_BigramLanguageModel_V2 (dense transformer fwd +
log-softmax CE loss), 8-core data-parallel SPMD.

Sharding: core c handles batch b=c//2, query-half h=c%2 (1024 of 2048 tokens).
Each core computes full-sequence K/V for its batch, its half's Q, causal
attention, the lm_head matmul over the full 32000 vocab for its 1024 tokens,
and fused log-softmax statistics (exp+row-sum on ScalarE). All matmuls bf16
with fp32 PSUM accumulation; lm_b applied exactly via a K=2 (hi+lo) ones-row
matmul. Causal masks are per-core *input data* so all 8 cores share one SPMD
program. Host does only sharding / casts / gathers and the final loss
reduction.
"""
import numpy as np
import ml_dtypes
from contextlib import ExitStack

import concourse.bass as bass
import concourse.tile as tile
from concourse import bacc, mybir
from concourse import library_config

BF16 = mybir.dt.bfloat16
F32 = mybir.dt.float32
I16 = mybir.dt.int16
AF = mybir.ActivationFunctionType

B, T, E, V = 4, 2048, 768, 32000
TQ = T // 2              # tokens per core
EC = E // 128            # 6 embedding chunks
VC = 500                 # vocab chunk width
NVC = V // VC            # 64 chunks
NQT = TQ // 128          # 8 query tiles per core
NST = T // 128           # 16 key tiles
SCALE = float(E) ** -0.5

_nbf = np.dtype(ml_dtypes.bfloat16)


def _build_program():
    nc = bacc.Bacc("TRN2", target_bir_lowering=False, debug=False)

    xe_d = nc.dram_tensor("xe", [T // 512, 128, EC, 512], BF16,
                          kind="ExternalInput")
    xqe_d = nc.dram_tensor("xqe", [TQ // 512, 128, EC, 512], BF16,
                          kind="ExternalInput")
    posf_d = nc.dram_tensor("posf", [T // 512, 128, EC, 512], BF16,
                          kind="ExternalInput")
    posq_d = nc.dram_tensor("posq", [TQ // 512, 128, EC, 512], BF16,
                          kind="ExternalInput")
    wq_d = nc.dram_tensor("wq", [128, EC, E], BF16, kind="ExternalInput")
    wk_d = nc.dram_tensor("wk", [128, EC, E], BF16, kind="ExternalInput")
    wv_d = nc.dram_tensor("wv", [128, EC, E], BF16, kind="ExternalInput")
    mask_d = nc.dram_tensor("mask", [NST, 128, TQ], BF16, kind="ExternalInput")
    bias_d = nc.dram_tensor("biasb", [128, V], F32, kind="ExternalInput")
    wtgt_d = nc.dram_tensor("wtgt", [128, EC, TQ], BF16, kind="ExternalInput")
    lmw_d = nc.dram_tensor("lmw", [NVC, 128, EC, VC], BF16, kind="ExternalInput")

    logits_d = nc.dram_tensor("logits", [TQ, V], F32, kind="ExternalOutput")
    stats_d = nc.dram_tensor("stats", [128, 16], F32, kind="ExternalOutput")

    with tile.TileContext(nc) as tc, ExitStack() as ctx:
        # ---- persistent pools ----
        pk = ctx.enter_context(tc.tile_pool(name="keep", bufs=1))
        kT = pk.tile([128, EC, T], BF16, tag="kT")
        qT = pk.tile([128, EC, TQ], BF16, tag="qT")
        v_sb = pk.tile([128, NST, E], BF16, tag="v")
        xaT = pk.tile([128, EC, TQ], BF16, tag="xaT")
        seacc = [pk.tile([128, NVC], F32, tag=f"seacc{qt}", name=f"seacc{qt}")
                 for qt in range(NQT)]
        onesd = pk.tile([128, 1], BF16, tag="onesd")
        nc.vector.memset(onesd[:], 1.0)
        ones1f = pk.tile([1, 128], F32, tag="ones1f")
        nc.vector.memset(ones1f[:], 1.0)

        # ---- phase E+P: embed & QKV projections ----
        with tc.tile_pool(name="emb", bufs=1) as pe, \
             tc.tile_pool(name="psA", bufs=4, space="PSUM") as psA:
            xTn, posfn, xqTn, posqn = [], [], [], []
            for n in range(T // 512):
                xt = pe.tile([128, EC, 512], BF16, tag=f"xT{n}", name=f"xT{n}")
                pf = pe.tile([128, EC, 512], BF16, tag=f"posf{n}", name=f"posf{n}")
                nc.scalar.dma_start(xt[:], xe_d.ap()[n])
                nc.scalar.dma_start(pf[:], posf_d.ap()[n])
                nc.vector.tensor_add(xt[:], xt[:], pf[:])
                xTn.append(xt)
                posfn.append(pf)
            for n in range(TQ // 512):
                xt = pe.tile([128, EC, 512], BF16, tag=f"xqT{n}", name=f"xqT{n}")
                pf = pe.tile([128, EC, 512], BF16, tag=f"posq{n}", name=f"posq{n}")
                nc.scalar.dma_start(xt[:], xqe_d.ap()[n])
                nc.scalar.dma_start(pf[:], posq_d.ap()[n])
                nc.vector.tensor_add(xt[:], xt[:], pf[:])
                xqTn.append(xt)
                posqn.append(pf)

            wq = pe.tile([128, EC, E], BF16, tag="wq")
            nc.scalar.dma_start(wq[:], wq_d.ap())
            wk = pe.tile([128, EC, E], BF16, tag="wk")
            nc.scalar.dma_start(wk[:], wk_d.ap())
            wv = pe.tile([128, EC, E], BF16, tag="wv")
            nc.scalar.dma_start(wv[:], wv_d.ap())

            # k^T [E, T] and q^T [E, TQ] in d-chunk layout; v natural [T, E]
            for n in range(T // 512):
                for m in range(EC):
                    ps = psA.tile([128, 512], F32, tag="psA")
                    for c in range(EC):
                        nc.tensor.matmul(ps[:], lhsT=wk[:, c, m * 128:(m + 1) * 128],
                                         rhs=xTn[n][:, c, :],
                                         start=(c == 0), stop=(c == EC - 1))
                    nc.vector.tensor_copy(kT[:, m, n * 512:(n + 1) * 512], ps[:])
                for s in range(4 * n, 4 * n + 4):
                    for hf in range(2):
                        ps = psA.tile([128, 384], F32, tag="psAv")
                        for c in range(EC):
                            nc.tensor.matmul(ps[:],
                                             lhsT=xTn[s // 4][:, c, (s % 4) * 128:(s % 4 + 1) * 128],
                                             rhs=wv[:, c, hf * 384:(hf + 1) * 384],
                                             start=(c == 0), stop=(c == EC - 1))
                        nc.vector.tensor_copy(v_sb[:, s, hf * 384:(hf + 1) * 384],
                                              ps[:])
            for n in range(TQ // 512):
                for m in range(EC):
                    ps = psA.tile([128, 512], F32, tag="psA")
                    for c in range(EC):
                        nc.tensor.matmul(ps[:], lhsT=wq[:, c, m * 128:(m + 1) * 128],
                                         rhs=xqTn[n][:, c, :],
                                         start=(c == 0), stop=(c == EC - 1))
                    nc.vector.tensor_copy(qT[:, m, n * 512:(n + 1) * 512], ps[:])

        # ---- phase A: causal attention, [s, q] orientation ----
        with tc.tile_pool(name="att", bufs=3) as pa, \
             tc.tile_pool(name="attm", bufs=3) as pm, \
             tc.tile_pool(name="attr", bufs=2) as pr, \
             tc.tile_pool(name="psB", bufs=1, space="PSUM") as psB:
            for g in range(2):                       # q-chunk of 512
                q0 = g * 512
                # sigma needed iff 128*s <= max over cores of last query
                n_sig = 12 if g == 0 else 16
                att = [psB.tile([128, 512], F32, tag=f"att{i}", name=f"att{i}")
                       for i in range(EC)]
                den = psB.tile([1, 512], F32, tag="den")
                for s in range(n_sig):
                    st = psB.tile([128, 512], F32, tag="st")
                    for c in range(EC):
                        nc.tensor.matmul(st[:], lhsT=kT[:, c, s * 128:(s + 1) * 128],
                                         rhs=qT[:, c, q0:q0 + 512],
                                         start=(c == 0), stop=(c == EC - 1))
                    pT = pa.tile([128, 512], BF16, tag="pT")
                    nc.scalar.activation(pT[:], st[:], AF.Exp, scale=SCALE)
                    # mask unless fully-valid for every core (h=0 bound)
                    if not (128 * s + 127 <= q0):
                        mk = pm.tile([128, 512], BF16, tag="mk")
                        nc.sync.dma_start(mk[:], mask_d.ap()[s, :, q0:q0 + 512])
                        nc.vector.tensor_mul(pT[:], pT[:], mk[:])
                    first, last = (s == 0), (s == n_sig - 1)
                    for i in range(EC):
                        nc.tensor.matmul(att[i][:],
                                         lhsT=v_sb[:, s, i * 128:(i + 1) * 128],
                                         rhs=pT[:], start=first, stop=last)
                    nc.tensor.matmul(den[:], lhsT=onesd[:], rhs=pT[:],
                                     start=first, stop=last)
                # normalize: xaT[:, :, q-slice] = att / den (bcast over partitions)
                rec = pr.tile([1, 512], F32, tag="rec")
                nc.vector.reciprocal(rec[:], den[:])
                rb_ps = psB.tile([128, 512], F32, tag="st")
                nc.tensor.matmul(rb_ps[:], lhsT=ones1f[:], rhs=rec[:],
                                 start=True, stop=True)
                rb = pr.tile([128, 512], F32, tag="rb")
                nc.vector.tensor_copy(rb[:], rb_ps[:])
                for i in range(EC):
                    nc.vector.tensor_mul(xaT[:, i, q0:q0 + 512], att[i][:], rb[:])

        # ---- phase L: lm_head + fused CE stats ----
        with tc.tile_pool(name="lmw", bufs=3) as pw, \
             tc.tile_pool(name="lmo", bufs=6) as po, \
             tc.tile_pool(name="lms", bufs=3) as pescr, \
             tc.tile_pool(name="psC", bufs=8, space="PSUM") as psC, \
             tc.tile_pool(name="st", bufs=1) as pst:
            stats = pst.tile([128, 16], F32, tag="stats")
            nc.vector.memset(stats[:], 0.0)
            wtg = pst.tile([128, EC, TQ], BF16, tag="wtg")
            nc.sync.dma_start(wtg[:], wtgt_d.ap())
            prod = pst.tile([128, TQ], F32, tag="prod")
            pacc = pst.tile([128, EC], F32, tag="pacc")
            for c in range(EC):
                nc.vector.tensor_mul(prod[:], xaT[:, c, :], wtg[:, c, :])
                nc.vector.reduce_sum(pacc[:, c:c + 1], prod[:],
                                     axis=mybir.AxisListType.X)
            nc.vector.reduce_sum(stats[:, 8:9], pacc[:], axis=mybir.AxisListType.X)
            for vc in range(NVC):
                wt = pw.tile([128, EC, VC], BF16, tag="wt")
                nc.sync.dma_start(wt[:], lmw_d.ap()[vc])
                bt = pw.tile([128, VC], F32, tag="bt")
                nc.sync.dma_start(bt[:], bias_d.ap()[:, vc * VC:(vc + 1) * VC])
                for qt in range(NQT):
                    ps = psC.tile([128, VC], F32, tag="lps")
                    for c in range(EC):
                        nc.tensor.matmul(ps[:],
                                         lhsT=xaT[:, c, qt * 128:(qt + 1) * 128],
                                         rhs=wt[:, c, :],
                                         start=(c == 0), stop=(c == EC - 1))
                    lsb = po.tile([128, VC], F32, tag="lsb")
                    nc.vector.tensor_add(lsb[:], ps[:], bt[:])
                    esc = pescr.tile([128, VC], BF16, tag="esc")
                    nc.scalar.activation(esc[:], lsb[:], AF.Exp,
                                         accum_out=seacc[qt][:, vc:vc + 1])
                    nc.sync.dma_start(
                        logits_d.ap()[qt * 128:(qt + 1) * 128, vc * VC:(vc + 1) * VC],
                        lsb[:])

            # stats tail: logsumexp per token + DMA out
            setot = pst.tile([128, NQT], F32, tag="setot")
            for qt in range(NQT):
                nc.vector.reduce_sum(setot[:, qt:qt + 1], seacc[qt][:],
                                     axis=mybir.AxisListType.X)
            nc.scalar.activation(stats[:, 0:NQT], setot[:], AF.Ln)
            nc.sync.dma_start(stats_d.ap(), stats[:])

    nc.compile()
    return nc


_NC_CACHE = None


def _get_nc():
    global _NC_CACHE
    if _NC_CACHE is None:
        _NC_CACHE = _build_program()
    return _NC_CACHE


def _wrap_idx(ix):
    """[n] int -> [128, n//16] int16 wrapped layout for dma_gather."""
    n = ix.shape[0]
    w = np.zeros((16, n // 16), dtype=np.int16)
    w[np.arange(n) % 16, np.arange(n) // 16] = ix.astype(np.int16)
    return np.tile(w, (8, 1))


def _chunked(w):
    """[E, N] -> [128, EC, N] with [p, c, n] = w[c*128+p, n]."""
    N = w.shape[1]
    return np.ascontiguousarray(w.reshape(EC, 128, N).transpose(1, 0, 2))


def make_in_maps(inputs):
    idx = np.asarray(inputs["idx"])
    target = np.asarray(inputs["target"])
    tok_emb = np.asarray(inputs["tok_emb"], dtype=np.float32)
    pos_emb = np.asarray(inputs["pos_emb"], dtype=np.float32)
    Wq = np.asarray(inputs["Wq"], dtype=np.float32)
    Wk = np.asarray(inputs["Wk"], dtype=np.float32)
    Wv = np.asarray(inputs["Wv"], dtype=np.float32)
    lm_W = np.asarray(inputs["lm_W"], dtype=np.float32)
    lm_b = np.asarray(inputs["lm_b"], dtype=np.float32)

    emb_bf = tok_emb.astype(_nbf)
    lmw_l = _chunked(lm_W.astype(_nbf))                      # [128, 6, V]
    lmw_cm = np.ascontiguousarray(
        lmw_l.reshape(128, EC, NVC, VC).transpose(2, 0, 1, 3))  # [NVC,128,6,VC]
    wq_l = _chunked(Wq.astype(_nbf))
    wk_l = _chunked(Wk.astype(_nbf))
    wv_l = _chunked(Wv.astype(_nbf))
    bias_bc = np.ascontiguousarray(
        np.broadcast_to(lm_b.astype(np.float32), (128, V)))
    posT = np.ascontiguousarray(
        pos_emb.astype(_nbf).reshape(T // 512, 512, EC, 128).transpose(0, 3, 2, 1))

    s_idx = 128 * np.arange(NST)[:, None, None] + np.arange(128)[None, :, None]

    in_maps, b_tgt_sums = [], []
    for c in range(8):
        b, h = c // 2, c % 2
        tgt = target[b, h * TQ:(h + 1) * TQ]
        mask = (s_idx <= (h * TQ + np.arange(TQ))[None, None, :]).astype(_nbf)
        xe = np.ascontiguousarray(
            emb_bf[idx[b]].reshape(T // 512, 512, EC, 128).transpose(0, 3, 2, 1))
        in_maps.append({
            "xe": xe,
            "xqe": np.ascontiguousarray(xe[2 * h:2 * h + 2]),
            "posf": posT,
            "posq": np.ascontiguousarray(posT[2 * h:2 * h + 2]),
            "wq": wq_l, "wk": wk_l, "wv": wv_l,
            "mask": mask,
            "biasb": bias_bc,
            "wtgt": np.ascontiguousarray(lmw_l[:, :, tgt]),
            "lmw": lmw_cm,
        })
        b_tgt_sums.append(float(lm_b[tgt].astype(np.float32).sum()))
    return in_maps, b_tgt_sums


def assemble(results, b_tgt_sums):
    logits = np.empty((B, T, V), dtype=np.float32)
    nll_sum = 0.0
    for c in range(8):
        b, h = c // 2, c % 2
        logits[b, h * TQ:(h + 1) * TQ] = results[c]["logits"]
        st = results[c]["stats"]
        nll_sum += float(st[:, 0:NQT].sum()) - float(st[:, 8].sum()) - b_tgt_sums[c]
    loss = np.float32(nll_sum / (B * T))
    return logits, loss


def kernel(**inputs):
    from concourse.bass_utils import run_bass_kernel_spmd
    nc = _get_nc()
    in_maps, b_tgt_sums = make_in_maps(inputs)
    res = run_bass_kernel_spmd(nc, in_maps, core_ids=list(range(8)))
    return assemble(res.results, b_tgt_sums)


# revision 9
# speedup vs baseline: 1.2618x; 1.0052x over previous
"""Trainium2 Bass kernel for nn_BigramLanguageModel_V2 (dense transformer fwd +
log-softmax CE loss), 8-core data-parallel SPMD.

Sharding: core c handles batch b=c//2, query-half h=c%2 (1024 of 2048 tokens).
Each core computes full-sequence K/V for its batch, its half's Q, causal
attention, the lm_head matmul over the full 32000 vocab for its 1024 tokens,
and fused log-softmax statistics (exp+row-sum on ScalarE). All matmuls bf16
with fp32 PSUM accumulation; lm_b applied exactly via a K=2 (hi+lo) ones-row
matmul. Causal masks are per-core *input data* so all 8 cores share one SPMD
program. Host does only sharding / casts / gathers and the final loss
reduction.
"""
import numpy as np
import ml_dtypes
from contextlib import ExitStack

import concourse.bass as bass
import concourse.tile as tile
from concourse import bacc, mybir
from concourse import library_config

BF16 = mybir.dt.bfloat16
F32 = mybir.dt.float32
I16 = mybir.dt.int16
AF = mybir.ActivationFunctionType

B, T, E, V = 4, 2048, 768, 32000
TQ = T // 2              # tokens per core
EC = E // 128            # 6 embedding chunks
VC = 500                 # vocab chunk width
NVC = V // VC            # 64 chunks
NQT = TQ // 128          # 8 query tiles per core
NST = T // 128           # 16 key tiles
SCALE = float(E) ** -0.5

_nbf = np.dtype(ml_dtypes.bfloat16)


def _build_program():
    nc = bacc.Bacc("TRN2", target_bir_lowering=False, debug=False)

    xe_d = nc.dram_tensor("xe", [T // 512, 128, EC, 512], BF16,
                          kind="ExternalInput")
    xqe_d = nc.dram_tensor("xqe", [TQ // 512, 128, EC, 512], BF16,
                          kind="ExternalInput")
    posf_d = nc.dram_tensor("posf", [T // 512, 128, EC, 512], BF16,
                          kind="ExternalInput")
    posq_d = nc.dram_tensor("posq", [TQ // 512, 128, EC, 512], BF16,
                          kind="ExternalInput")
    wq_d = nc.dram_tensor("wq", [128, EC, E], BF16, kind="ExternalInput")
    wk_d = nc.dram_tensor("wk", [128, EC, E], BF16, kind="ExternalInput")
    wv_d = nc.dram_tensor("wv", [128, EC, E], BF16, kind="ExternalInput")
    mask_d = nc.dram_tensor("mask", [NST, 128, TQ], BF16, kind="ExternalInput")
    bias_d = nc.dram_tensor("biasb", [128, V], F32, kind="ExternalInput")
    wtgt_d = nc.dram_tensor("wtgt", [128, EC, TQ], BF16, kind="ExternalInput")
    lmw_d = nc.dram_tensor("lmw", [NVC, 128, EC, VC], BF16, kind="ExternalInput")

    logits_d = nc.dram_tensor("logits", [TQ, V], F32, kind="ExternalOutput")
    stats_d = nc.dram_tensor("stats", [128, 16], F32, kind="ExternalOutput")

    with tile.TileContext(nc) as tc, ExitStack() as ctx:
        # ---- persistent pools ----
        pk = ctx.enter_context(tc.tile_pool(name="keep", bufs=1))
        kT = pk.tile([128, EC, T], BF16, tag="kT")
        qT = pk.tile([128, EC, TQ], BF16, tag="qT")
        v_sb = pk.tile([128, NST, E], BF16, tag="v")
        xaT = pk.tile([128, EC, TQ], BF16, tag="xaT")
        seacc = [pk.tile([128, NVC], F32, tag=f"seacc{qt}", name=f"seacc{qt}")
                 for qt in range(NQT)]
        onesd = pk.tile([128, 1], BF16, tag="onesd")
        nc.vector.memset(onesd[:], 1.0)
        ones1f = pk.tile([1, 128], F32, tag="ones1f")
        nc.vector.memset(ones1f[:], 1.0)

        # ---- phase E+P: embed & QKV projections ----
        with tc.tile_pool(name="emb", bufs=1) as pe, \
             tc.tile_pool(name="psA", bufs=4, space="PSUM") as psA:
            xTn, posfn, xqTn, posqn = [], [], [], []
            for n in range(T // 512):
                xt = pe.tile([128, EC, 512], BF16, tag=f"xT{n}", name=f"xT{n}")
                pf = pe.tile([128, EC, 512], BF16, tag=f"posf{n}", name=f"posf{n}")
                nc.scalar.dma_start(xt[:], xe_d.ap()[n])
                nc.scalar.dma_start(pf[:], posf_d.ap()[n])
                nc.vector.tensor_add(xt[:], xt[:], pf[:])
                xTn.append(xt)
                posfn.append(pf)
            for n in range(TQ // 512):
                xt = pe.tile([128, EC, 512], BF16, tag=f"xqT{n}", name=f"xqT{n}")
                pf = pe.tile([128, EC, 512], BF16, tag=f"posq{n}", name=f"posq{n}")
                nc.scalar.dma_start(xt[:], xqe_d.ap()[n])
                nc.scalar.dma_start(pf[:], posq_d.ap()[n])
                nc.vector.tensor_add(xt[:], xt[:], pf[:])
                xqTn.append(xt)
                posqn.append(pf)

            wq = pe.tile([128, EC, E], BF16, tag="wq")
            nc.scalar.dma_start(wq[:], wq_d.ap())
            wk = pe.tile([128, EC, E], BF16, tag="wk")
            nc.scalar.dma_start(wk[:], wk_d.ap())
            wv = pe.tile([128, EC, E], BF16, tag="wv")
            nc.scalar.dma_start(wv[:], wv_d.ap())

            # k^T [E, T] and q^T [E, TQ] in d-chunk layout; v natural [T, E]
            for n in range(T // 512):
                for m in range(EC):
                    ps = psA.tile([128, 512], F32, tag="psA")
                    for c in range(EC):
                        nc.tensor.matmul(ps[:], lhsT=wk[:, c, m * 128:(m + 1) * 128],
                                         rhs=xTn[n][:, c, :],
                                         start=(c == 0), stop=(c == EC - 1))
                    nc.vector.tensor_copy(kT[:, m, n * 512:(n + 1) * 512], ps[:])
                for s in range(4 * n, 4 * n + 4):
                    for hf in range(2):
                        ps = psA.tile([128, 384], F32, tag="psAv")
                        for c in range(EC):
                            nc.tensor.matmul(ps[:],
                                             lhsT=xTn[s // 4][:, c, (s % 4) * 128:(s % 4 + 1) * 128],
                                             rhs=wv[:, c, hf * 384:(hf + 1) * 384],
                                             start=(c == 0), stop=(c == EC - 1))
                        nc.vector.tensor_copy(v_sb[:, s, hf * 384:(hf + 1) * 384],
                                              ps[:])
            for n in range(TQ // 512):
                for m in range(EC):
                    ps = psA.tile([128, 512], F32, tag="psA")
                    for c in range(EC):
                        nc.tensor.matmul(ps[:], lhsT=wq[:, c, m * 128:(m + 1) * 128],
                                         rhs=xqTn[n][:, c, :],
                                         start=(c == 0), stop=(c == EC - 1))
                    nc.vector.tensor_copy(qT[:, m, n * 512:(n + 1) * 512], ps[:])

        # ---- phase A: causal attention, [s, q] orientation ----
        psB = ctx.enter_context(tc.tile_pool(name="psB", bufs=1, space="PSUM"))
        with tc.tile_pool(name="att", bufs=3) as pa, \
             tc.tile_pool(name="attm", bufs=3) as pm, \
             tc.tile_pool(name="attr", bufs=2) as pr:
            for g in range(2):                       # q-chunk of 512
                q0 = g * 512
                # sigma needed iff 128*s <= max over cores of last query
                n_sig = 12 if g == 0 else 16
                att = [psB.tile([128, 512], F32, tag=f"att{i}", name=f"att{i}")
                       for i in range(EC)]
                den = psB.tile([1, 512], F32, tag="den")
                for s in range(n_sig):
                    st = psB.tile([128, 512], F32, tag="st")
                    for c in range(EC):
                        nc.tensor.matmul(st[:], lhsT=kT[:, c, s * 128:(s + 1) * 128],
                                         rhs=qT[:, c, q0:q0 + 512],
                                         start=(c == 0), stop=(c == EC - 1))
                    pT = pa.tile([128, 512], BF16, tag="pT")
                    nc.scalar.activation(pT[:], st[:], AF.Exp, scale=SCALE)
                    # mask unless fully-valid for every core (h=0 bound)
                    if not (128 * s + 127 <= q0):
                        mk = pm.tile([128, 512], BF16, tag="mk")
                        nc.sync.dma_start(mk[:], mask_d.ap()[s, :, q0:q0 + 512])
                        nc.vector.tensor_mul(pT[:], pT[:], mk[:])
                    first, last = (s == 0), (s == n_sig - 1)
                    for i in range(EC):
                        nc.tensor.matmul(att[i][:],
                                         lhsT=v_sb[:, s, i * 128:(i + 1) * 128],
                                         rhs=pT[:], start=first, stop=last)
                    nc.tensor.matmul(den[:], lhsT=onesd[:], rhs=pT[:],
                                     start=first, stop=last)
                # normalize: xaT[:, :, q-slice] = att / den (bcast over partitions)
                rec = pr.tile([1, 512], F32, tag="rec")
                nc.vector.reciprocal_approx_fast(rec[:], den[:])
                rb_ps = psB.tile([128, 512], F32, tag="st")
                nc.tensor.matmul(rb_ps[:], lhsT=ones1f[:], rhs=rec[:],
                                 start=True, stop=True)
                rb = pr.tile([128, 512], F32, tag="rb")
                nc.vector.tensor_copy(rb[:], rb_ps[:])
                for i in range(EC):
                    nc.vector.tensor_mul(xaT[:, i, q0:q0 + 512], att[i][:], rb[:])

        # ---- phase L: lm_head + fused CE stats ----
        with tc.tile_pool(name="lmw", bufs=3) as pw, \
             tc.tile_pool(name="lmo", bufs=6) as po, \
             tc.tile_pool(name="lms", bufs=3) as pescr, \
             tc.tile_pool(name="st", bufs=1) as pst:
            stats = pst.tile([128, 16], F32, tag="stats")
            nc.vector.memset(stats[:], 0.0)
            wtg = pst.tile([128, EC, TQ], BF16, tag="wtg")
            nc.sync.dma_start(wtg[:], wtgt_d.ap())
            prod = pst.tile([128, TQ], F32, tag="prod")
            pacc = pst.tile([128, EC], F32, tag="pacc")
            for c in range(EC):
                nc.vector.tensor_mul(prod[:], xaT[:, c, :], wtg[:, c, :])
                nc.vector.reduce_sum(pacc[:, c:c + 1], prod[:],
                                     axis=mybir.AxisListType.X)
            nc.vector.reduce_sum(stats[:, 8:9], pacc[:], axis=mybir.AxisListType.X)
            for vc in range(NVC):
                wt = pw.tile([128, EC, VC], BF16, tag="wt")
                nc.sync.dma_start(wt[:], lmw_d.ap()[vc])
                bt = pw.tile([128, VC], F32, tag="bt")
                nc.sync.dma_start(bt[:], bias_d.ap()[:, vc * VC:(vc + 1) * VC])
                for qt in range(NQT):
                    _tg = ["den", "st", "att0", "att1", "att2", "att3",
                           "att4", "att5"][(vc * NQT + qt) % 8]
                    ps = psB.tile([128, VC], F32, tag=_tg, name=f"lps_{vc}_{qt}")
                    for c in range(EC):
                        nc.tensor.matmul(ps[:],
                                         lhsT=xaT[:, c, qt * 128:(qt + 1) * 128],
                                         rhs=wt[:, c, :],
                                         start=(c == 0), stop=(c == EC - 1))
                    lsb = po.tile([128, VC], F32, tag="lsb")
                    nc.vector.tensor_add(lsb[:], ps[:], bt[:])
                    esc = pescr.tile([128, VC], BF16, tag="esc")
                    nc.scalar.activation(esc[:], lsb[:], AF.Exp,
                                         accum_out=seacc[qt][:, vc:vc + 1])
                    nc.sync.dma_start(
                        logits_d.ap()[qt * 128:(qt + 1) * 128, vc * VC:(vc + 1) * VC],
                        lsb[:])

            # stats tail: logsumexp per token + DMA out
            setot = pst.tile([128, NQT], F32, tag="setot")
            for qt in range(NQT):
                nc.vector.reduce_sum(setot[:, qt:qt + 1], seacc[qt][:],
                                     axis=mybir.AxisListType.X)
            nc.scalar.activation(stats[:, 0:NQT], setot[:], AF.Ln)
            nc.sync.dma_start(stats_d.ap(), stats[:])

    nc.compile()
    return nc


_NC_CACHE = None


def _get_nc():
    global _NC_CACHE
    if _NC_CACHE is None:
        _NC_CACHE = _build_program()
    return _NC_CACHE


def _wrap_idx(ix):
    """[n] int -> [128, n//16] int16 wrapped layout for dma_gather."""
    n = ix.shape[0]
    w = np.zeros((16, n // 16), dtype=np.int16)
    w[np.arange(n) % 16, np.arange(n) // 16] = ix.astype(np.int16)
    return np.tile(w, (8, 1))


def _chunked(w):
    """[E, N] -> [128, EC, N] with [p, c, n] = w[c*128+p, n]."""
    N = w.shape[1]
    return np.ascontiguousarray(w.reshape(EC, 128, N).transpose(1, 0, 2))


def make_in_maps(inputs):
    idx = np.asarray(inputs["idx"])
    target = np.asarray(inputs["target"])
    tok_emb = np.asarray(inputs["tok_emb"], dtype=np.float32)
    pos_emb = np.asarray(inputs["pos_emb"], dtype=np.float32)
    Wq = np.asarray(inputs["Wq"], dtype=np.float32)
    Wk = np.asarray(inputs["Wk"], dtype=np.float32)
    Wv = np.asarray(inputs["Wv"], dtype=np.float32)
    lm_W = np.asarray(inputs["lm_W"], dtype=np.float32)
    lm_b = np.asarray(inputs["lm_b"], dtype=np.float32)

    emb_bf = tok_emb.astype(_nbf)
    lmw_l = _chunked(lm_W.astype(_nbf))                      # [128, 6, V]
    lmw_cm = np.ascontiguousarray(
        lmw_l.reshape(128, EC, NVC, VC).transpose(2, 0, 1, 3))  # [NVC,128,6,VC]
    wq_l = _chunked(Wq.astype(_nbf))
    wk_l = _chunked(Wk.astype(_nbf))
    wv_l = _chunked(Wv.astype(_nbf))
    bias_bc = np.ascontiguousarray(
        np.broadcast_to(lm_b.astype(np.float32), (128, V)))
    posT = np.ascontiguousarray(
        pos_emb.astype(_nbf).reshape(T // 512, 512, EC, 128).transpose(0, 3, 2, 1))

    s_idx = 128 * np.arange(NST)[:, None, None] + np.arange(128)[None, :, None]

    in_maps, b_tgt_sums = [], []
    for c in range(8):
        b, h = c // 2, c % 2
        tgt = target[b, h * TQ:(h + 1) * TQ]
        mask = (s_idx <= (h * TQ + np.arange(TQ))[None, None, :]).astype(_nbf)
        xe = np.ascontiguousarray(
            emb_bf[idx[b]].reshape(T // 512, 512, EC, 128).transpose(0, 3, 2, 1))
        in_maps.append({
            "xe": xe,
            "xqe": np.ascontiguousarray(xe[2 * h:2 * h + 2]),
            "posf": posT,
            "posq": np.ascontiguousarray(posT[2 * h:2 * h + 2]),
            "wq": wq_l, "wk": wk_l, "wv": wv_l,
            "mask": mask,
            "biasb": bias_bc,
            "wtgt": np.ascontiguousarray(lmw_l[:, :, tgt]),
            "lmw": lmw_cm,
        })
        b_tgt_sums.append(float(lm_b[tgt].astype(np.float32).sum()))
    return in_maps, b_tgt_sums


def assemble(results, b_tgt_sums):
    logits = np.empty((B, T, V), dtype=np.float32)
    nll_sum = 0.0
    for c in range(8):
        b, h = c // 2, c % 2
        logits[b, h * TQ:(h + 1) * TQ] = results[c]["logits"]
        st = results[c]["stats"]
        nll_sum += float(st[:, 0:NQT].sum()) - float(st[:, 8].sum()) - b_tgt_sums[c]
    loss = np.float32(nll_sum / (B * T))
    return logits, loss


def kernel(**inputs):
    from concourse.bass_utils import run_bass_kernel_spmd
    nc = _get_nc()
    in_maps, b_tgt_sums = make_in_maps(inputs)
    res = run_bass_kernel_spmd(nc, in_maps, core_ids=list(range(8)))
    return assemble(res.results, b_tgt_sums)
